# revision 35
# baseline (speedup 1.0000x reference)
"""GAT 2-layer kernel for 8 Trainium2 NeuronCores (Bass/Tile).

Strategy (graph/data parallel per the dst-partition sharding hint):
  - Nodes in packed space of 50176 = 8*6272 rows. Core c owns packed rows
    [6272c, 6272(c+1)) = 49 dst tiles of 128. Each core's xT input has its
    own shard's columns rotated to the front, so "local" is tiles 0..48 in
    every core's identical SPMD program.
  - Projection (replicated): xw = x @ [W1 | W1@Asrc_blk | W1@Adst_blk]
    (264 cols). Feature rows are written to a DRAM table XW[50176, 384]
    bf16: 256 feat + 4 fp32 alpha_src (bitcast to 8 bf16 slots) + pad.
    Rows are stored in a per-partition-contiguous permuted order
    (rowidx = b*3584 + p*28 + i for node n = (b*28+i)*128 + p) so each
    DMA descriptor writes 28 rows (21.5KB) contiguously; gather indices
    are permuted to match on the host.
  - Layer-1 edges: grouped by dst tile, split by rowidx half (int16
    limit), padded to 128-edge chunks on a shared cross-core schedule.
    Per chunk: dma_gather 768B rows (feat + alpha_src); alpha_dst per
    edge via one-hot matmul; p = exp(leaky_relu(als+ad)); segment
    softmax folded into one matmul of [p*feat | p] against dst one-hot.
  - xw2 = relu(S/denom + b1) @ w2 per local node; AllGather (bf16,
    12.5KB/core); then a 64-way 2-shift replica table XW2S[25088, 128]
    is built in DRAM (row j2*392+q = xw2[128q+2*j2 : +128]) so the
    layer-2 per-edge scalar gather lands each edge's value at column
    src%2 of its 256B row - a 2-way parity select instead of a 128-way
    one-hot, and the gather spreads over 6.4MB instead of hammering a
    100KB table. Single int16 half (25088 < 32768).
  - Layer-2 edges: dst-tile grouping keyed by row index; T2 one-hots
    for the per-edge dst value are built on the scalar engine as
    relu(1 - |dstrel_bcast - p|) to offload the saturated vector unit.
  - Gathers round-robin over 4 SWDGE queues (4 Q7 cpu pairs + rings);
    XW is split XWlo/XWhi at a batch-aligned int16 boundary.
"""
import numpy as np
import ml_dtypes

P = 128
N = 50000
FEAT = 256
HID = 64
HEADS = 4
NCORES = 8
TILES = 49                 # dst tiles per core
SHARD = TILES * P          # 6272 packed nodes per core
NPACK = NCORES * SHARD     # 50176
LOSPLIT1 = 32256           # L1 split: batches 0-8 (batch-aligned for XWlo/XWhi)
LOSPLIT2 = 32768           # L2 split (XW2S shifted table)
NTILES_GLOBAL = NPACK // P # 392 node tiles
NB = 28                    # projection tiles per batch
NBATCH = NTILES_GLOBAL // NB  # 14
ROWLEN = 384               # XW row length in bf16 elems (768B)
CMAX = 26                 # chunks per processing round
NEG_SLOPE = 0.2
EPS = 1e-16

bf16 = ml_dtypes.bfloat16


def _cdiv(a, b):
    return -(-a // b)


def _rowidx(n):
    """Permuted XW row index for rotated-space node n (vectorized)."""
    t = n // P
    p = n % P
    b = t // NB
    i = t % NB
    return b * (NB * P) + p * NB + i


def _shiftidx(s):
    """XW2S row index for global packed node s."""
    return (s % P) * NTILES_GLOBAL + s // P


# ----------------------------------------------------------------------------
# host-side edge scheduling
# ----------------------------------------------------------------------------

def _sched_edges(key, tile, extra, ncores_masks):
    """Shared (tile, half) chunk schedule over cores for one key split.

    key: per-edge half (0/1); tile: per-edge dst tile; extra: list of
    per-edge arrays to reorder alongside. Returns (sched [TILES,2],
    per-core list of dicts with 'idx' (position within half, padded),
    reordered extras padded, each padded to the shared schedule).
    """
    counts = np.zeros((NCORES, TILES, 2), np.int64)
    ordered = []
    for c, m in enumerate(ncores_masks):
        tc, kc = tile[m], key[m]
        kk = tc * 2 + kc
        order = np.argsort(kk, kind="stable")
        ordered.append((m, order, kk[order]))
        counts[c] = np.bincount(kk, minlength=TILES * 2).reshape(TILES, 2)
    sched = _cdiv(counts, P).max(axis=0)
    return sched, counts, ordered


def _build_schedules(edge_index):
    """Build layer-1 (rowidx halves) and layer-2 (shiftidx halves) edge
    schedules. Returns (sched1, sched2, per_core list of dicts)."""
    src = np.concatenate([np.asarray(edge_index[0]),
                          np.arange(N, dtype=np.int64)]).astype(np.int64)
    dst = np.concatenate([np.asarray(edge_index[1]),
                          np.arange(N, dtype=np.int64)]).astype(np.int64)
    core = dst // SHARD
    tile = (dst % SHARD) // P

    per_core = []
    sched1 = np.zeros((TILES, 2), np.int64)
    sched2 = np.zeros((TILES, 2), np.int64)
    data = []
    for c in range(NCORES):
        m = core == c
        sc, dc, tc = src[m], dst[m], tile[m]
        rsrc = (sc - c * SHARD) % NPACK
        ridx = _rowidx(rsrc)
        half1 = (ridx >= LOSPLIT1).astype(np.int64)
        # L2: 2-shift table, row (j//2)*392 + q, parity selects col 0/1
        sidx = ((sc % P) // 2) * NTILES_GLOBAL + sc // P
        half2 = np.zeros_like(half1)
        data.append((sc, dc, tc, ridx, half1, sidx, half2))

    # shared schedules (max over cores)
    c1 = np.zeros((NCORES, TILES, 2), np.int64)
    c2 = np.zeros((NCORES, TILES, 2), np.int64)
    for c in range(NCORES):
        sc, dc, tc, ridx, half1, sidx, half2 = data[c]
        c1[c] = np.bincount(tc * 2 + half1, minlength=TILES * 2).reshape(TILES, 2)
        c2[c] = np.bincount(tc * 2 + half2, minlength=TILES * 2).reshape(TILES, 2)
    sched1 = _cdiv(c1, P).max(axis=0)
    sched2 = _cdiv(c2, P).max(axis=0)

    for c in range(NCORES):
        sc, dc, tc, ridx, half1, sidx, half2 = data[c]

        def pack(keyhalf, counts, sched, val, aux, padlo, padhi):
            """Order by (tile, half); pad each (tile,half) run to its
            scheduled chunk count. Returns (val_p, dstrel_p, aux_p)."""
            kk = tc * 2 + keyhalf
            order = np.argsort(kk, kind="stable")
            v_s, d_s = val[order], (dc % P)[order]
            a_s = aux[order] if aux is not None else None
            starts = np.zeros(TILES * 2 + 1, np.int64)
            np.cumsum(counts.reshape(-1), out=starts[1:])
            total = int(sched.sum())
            vp = np.zeros(total * P, np.int64)
            dp = np.full(total * P, -1.0, np.float32)
            ap = np.zeros(total * P, np.float32)
            pos = 0
            for t in range(TILES):
                for h in range(2):
                    k = t * 2 + h
                    cnt = int(counts[t, h])
                    n_ch = int(sched[t, h])
                    sl = slice(starts[k], starts[k] + cnt)
                    vp[pos: pos + cnt] = v_s[sl]
                    if h == 1:
                        vp[pos + cnt: pos + n_ch * P] = padhi
                    else:
                        vp[pos + cnt: pos + n_ch * P] = padlo
                    dp[pos: pos + cnt] = d_s[sl]
                    if a_s is not None:
                        ap[pos: pos + cnt] = a_s[sl]
                    pos += n_ch * P
            return vp, dp, ap

        v1, d1, _ = pack(half1, c1[c], sched1, ridx, None, 0, LOSPLIT1)
        v2, d2, p2 = pack(half2, c2[c], sched2, sidx,
                          (sc % 2).astype(np.float32), 0, LOSPLIT2)
        per_core.append(dict(v1=v1, d1=d1, v2=v2, d2=d2, p2=p2))
    return sched1, sched2, per_core


def _wrap_idx(idx16):
    """[n] int16 -> [128, n/16] wrapped (i at [i%16, i//16]) + 8x replicated."""
    a = idx16.reshape(-1, 16).T
    return np.tile(a, (8, 1)).copy()


def _split_halves(vp, sched, losplit):
    """Split padded per-edge values into lo/hi index tables (per sched)."""
    total = vp.size // P
    vmat = vp.reshape(total, P)
    lo_list, hi_list = [], []
    pos = 0
    for t in range(TILES):
        c_lo, c_hi = int(sched[t, 0]), int(sched[t, 1])
        lo_list.append(vmat[pos: pos + c_lo].reshape(-1))
        hi_list.append(vmat[pos + c_lo: pos + c_lo + c_hi].reshape(-1) - losplit)
        pos += c_lo + c_hi
    lo = np.concatenate(lo_list) if lo_list else np.zeros(0, np.int64)
    hi = np.concatenate(hi_list) if hi_list else np.zeros(0, np.int64)
    return lo.astype(np.int16), hi.astype(np.int16)


def _host_arrays(inputs):
    x = np.asarray(inputs["x"], np.float32)
    ei = np.asarray(inputs["edge_index"])
    w1 = np.asarray(inputs["w1"], np.float32)
    a_src1 = np.asarray(inputs["a_src1"], np.float32)
    a_dst1 = np.asarray(inputs["a_dst1"], np.float32)
    b1 = np.asarray(inputs["b1"], np.float32)
    w2 = np.asarray(inputs["w2"], np.float32)

    sched1, sched2, per_core = _build_schedules(ei)

    xT = np.zeros((FEAT, NPACK), bf16)
    xT[:, :N] = x.T.astype(bf16)

    # w1a = [W1 | W1@Asrc_blk | W1@Adst_blk]  -> [256, 264]
    a_src_blk = np.zeros((FEAT, HEADS), np.float32)
    a_dst_blk = np.zeros((FEAT, HEADS), np.float32)
    for h in range(HEADS):
        a_src_blk[h * HID:(h + 1) * HID, h] = a_src1[h]
        a_dst_blk[h * HID:(h + 1) * HID, h] = a_dst1[h]
    w1a = np.concatenate([w1, w1 @ a_src_blk, w1 @ a_dst_blk], axis=1)

    shared = dict(
        w1a_bf=w1a.reshape(2, P, FEAT + 2 * HEADS).astype(bf16),
        b1=b1.astype(np.float32),
        w2row=w2.reshape(-1).astype(np.float32),
        a_src2=float(np.asarray(inputs["a_src2"]).reshape(())),
        a_dst2=float(np.asarray(inputs["a_dst2"]).reshape(())),
        b2=float(np.asarray(inputs["b2"]).reshape(())),
        sched1=sched1,
        sched2=sched2,
    )

    cores = []
    for c in range(NCORES):
        pc = per_core[c]
        lo1, hi1 = _split_halves(pc["v1"], sched1, LOSPLIT1)
        lo2, hi2 = _split_halves(pc["v2"], sched2, LOSPLIT2)
        ct1 = pc["v1"].size // P
        ct2 = pc["v2"].size // P
        cores.append(dict(
            xT=np.roll(xT, -c * SHARD, axis=1).copy(),
            idx1_lo=_wrap_idx(lo1), idx1_hi=_wrap_idx(hi1) if hi1.size else np.zeros((P, 8), np.int16),
            idx2_lo=_wrap_idx(lo2), idx2_hi=_wrap_idx(hi2) if hi2.size else np.zeros((P, 8), np.int16),
            dstpc1=pc["d1"].reshape(ct1, P).T.astype(bf16).copy(),
            dstrow1=pc["d1"].reshape(1, ct1 * P).astype(bf16),
            dstpc2=pc["d2"].reshape(ct2, P).T.astype(bf16).copy(),
            dstrow2=pc["d2"].reshape(1, ct2 * P).astype(bf16),
            parpc2=pc["p2"].reshape(ct2, P).T.astype(bf16).copy(),
            v1=pc["v1"], v2=pc["v2"], p2=pc["p2"],
        ))
    return shared, cores


# ----------------------------------------------------------------------------
# numpy simulation of the exact device pipeline (layout validation)
# ----------------------------------------------------------------------------

def _simulate(shared, cores):
    f32 = np.float32
    sched1, sched2 = shared["sched1"], shared["sched2"]
    w1a = np.asarray(shared["w1a_bf"], f32).reshape(FEAT, FEAT + 2 * HEADS)
    b1 = shared["b1"]
    w2row = shared["w2row"]
    a2s, a2d, b2 = shared["a_src2"], shared["a_dst2"], shared["b2"]

    xw2_cores = []
    for c in range(NCORES):
        m = cores[c]
        xTc = np.asarray(m["xT"], f32)
        proj = xTc.T @ w1a                      # [NPACK, 264] fp32
        feat = proj[:, :FEAT].astype(bf16).astype(f32)
        als = proj[:, FEAT:FEAT + HEADS].astype(bf16).astype(f32)
        ad_loc = proj[:TILES * P, FEAT + HEADS:].astype(bf16).astype(f32)

        # XW table in permuted row order (row _rowidx(n) = node n)
        # gather for padded v1 (rowidx values)
        v1 = m["v1"]
        inv = np.empty(NPACK, np.int64)
        inv[_rowidx(np.arange(NPACK))] = np.arange(NPACK)
        srcs = inv[v1]                          # node ids per padded edge
        dstrel = m["dstpc1"].T.reshape(-1)

        Gf = feat[srcs]                         # [Etot, 256]
        Ga = als[srcs]                          # [Etot, 4] fp32
        S = np.zeros((TILES * P, FEAT + HEADS), f32)
        pos = 0
        for t in range(TILES):
            nch = int(sched1[t].sum())
            sl = slice(pos * P, (pos + nch) * P)
            pos += nch
            dr = dstrel[sl]
            onehot = (dr[:, None] == np.arange(P)[None, :]).astype(f32)
            ad_e = onehot @ ad_loc[t * P:(t + 1) * P]   # [e,4]
            ev = Ga[sl] + ad_e
            lrl = np.where(ev > 0, ev, NEG_SLOPE * ev)
            pv = np.exp(lrl).astype(bf16).astype(f32)
            msg = (Gf[sl].reshape(-1, HEADS, HID)
                   * pv[:, :, None]).reshape(-1, FEAT).astype(bf16).astype(f32)
            S[t * P:(t + 1) * P] += onehot.T @ np.concatenate([msg, pv], 1)
        den = S[:, FEAT:] + EPS
        h1 = S[:, :FEAT] / np.repeat(den, HID, axis=1) + b1
        h1 = np.maximum(h1, 0.0).astype(bf16).astype(f32)
        xw2_cores.append(h1 @ w2row)
    xw2_pack = np.concatenate(xw2_cores).astype(bf16).astype(f32)

    # XW2S shifted table: row j*392+q col k = xw2_pad[128q + j + k]
    xw2_pad = np.concatenate([xw2_pack, np.zeros(P, f32)])
    outs = []
    for c in range(NCORES):
        m = cores[c]
        v2 = m["v2"]                            # 2-shift row idx per edge
        j2, q = v2 // NTILES_GLOBAL, v2 % NTILES_GLOBAL
        par = m["p2"].astype(np.int64)
        xs = xw2_pad[q * P + 2 * j2 + par]      # col `par` of gathered row
        dstrel = m["dstpc2"].T.reshape(-1)
        xw2loc = xw2_pack[c * SHARD:(c + 1) * SHARD]
        S2 = np.zeros((TILES * P, 2), f32)
        pos = 0
        for t in range(TILES):
            nch = int(sched2[t].sum())
            sl = slice(pos * P, (pos + nch) * P)
            pos += nch
            dr = dstrel[sl]
            onehot = (dr[:, None] == np.arange(P)[None, :]).astype(f32)
            xd_e = onehot @ xw2loc[t * P:(t + 1) * P, None].astype(bf16).astype(f32)
            e2 = a2s * xs[sl] + a2d * xd_e[:, 0]
            lr2 = np.where(e2 > 0, e2, NEG_SLOPE * e2)
            p2 = np.exp(lr2)
            m2 = np.stack([p2 * xs[sl], p2], 1).astype(bf16).astype(f32)
            S2[t * P:(t + 1) * P] += onehot.T @ m2
        outs.append(S2[:, 0] / (S2[:, 1] + EPS) + b2)
    return np.concatenate(outs)[:N].reshape(N, 1).astype(f32)


def kernel_sim(**inputs):
    shared, cores = _host_arrays(inputs)
    return _simulate(shared, cores)


# ----------------------------------------------------------------------------
# device program
# ----------------------------------------------------------------------------

def _build_program(shared, n1lo, n1hi, ct1, n2lo, n2hi, ct2):
    import concourse.bacc as bacc
    import concourse.tile as tile
    import concourse.mybir as mybir

    sched1, sched2 = shared["sched1"], shared["sched2"]
    dt = mybir.dt
    AF = mybir.ActivationFunctionType
    OP = mybir.AluOpType
    W1AC = FEAT + 2 * HEADS  # 264

    nc = bacc.Bacc(None, target_bir_lowering=False, num_swdge_queues=4)
    qrr = [0]  # round-robin SWDGE queue selector for gathers

    def next_q():
        qrr[0] = (qrr[0] + 1) % 4
        return qrr[0]

    # ---- parameters ----
    xT_d = nc.declare_dram_parameter("xT", [FEAT, NPACK], dt.bfloat16, isOutput=False)
    w1a_d = nc.declare_dram_parameter("w1a", [2, P, W1AC], dt.bfloat16, isOutput=False)
    b1_d = nc.declare_dram_parameter("b1", [FEAT], dt.float32, isOutput=False)
    w2_d = nc.declare_dram_parameter("w2row", [FEAT], dt.float32, isOutput=False)
    i1lo_d = nc.declare_dram_parameter("idx1_lo", [P, n1lo * 8], dt.int16, isOutput=False)
    i1hi_d = nc.declare_dram_parameter("idx1_hi", [P, max(n1hi, 1) * 8], dt.int16, isOutput=False)
    i2lo_d = nc.declare_dram_parameter("idx2_lo", [P, n2lo * 8], dt.int16, isOutput=False)
    i2hi_d = nc.declare_dram_parameter("idx2_hi", [P, max(n2hi, 1) * 8], dt.int16, isOutput=False)
    dp1_d = nc.declare_dram_parameter("dstpc1", [P, ct1], dt.bfloat16, isOutput=False)
    dr1_d = nc.declare_dram_parameter("dstrow1", [1, ct1 * P], dt.bfloat16, isOutput=False)
    dp2_d = nc.declare_dram_parameter("dstpc2", [P, ct2], dt.bfloat16, isOutput=False)
    dr2_d = nc.declare_dram_parameter("dstrow2", [1, ct2 * P], dt.bfloat16, isOutput=False)
    pp2_d = nc.declare_dram_parameter("parpc2", [P, ct2], dt.bfloat16, isOutput=False)
    out_d = nc.declare_dram_parameter("out", [SHARD, 1], dt.float32, isOutput=True)

    XWlo = nc.dram_tensor("XWlo", [LOSPLIT1, ROWLEN], dt.bfloat16)
    XWhi = nc.dram_tensor("XWhi", [NPACK - LOSPLIT1, ROWLEN], dt.bfloat16)
    NSHIFT = P // 2  # 64 2-shift replicas
    XW2S = nc.dram_tensor("XW2S", [NSHIFT * NTILES_GLOBAL, P], dt.bfloat16)
    NBLO = LOSPLIT1 // (NB * P)  # 9 batches in XWlo

    a2s, a2d, b2 = shared["a_src2"], shared["a_dst2"], shared["b2"]

    with tile.TileContext(nc) as tc:
      with (
          tc.tile_pool(name="const", bufs=1) as cpool,
          tc.tile_pool(name="dram", bufs=1, space="DRAM") as dpool,
      ):
        # ---- persistent constants / state ----
        it32 = cpool.tile([P, 1, P], dt.int32)
        nc.gpsimd.iota(it32[:, 0, :], [[1, P]], channel_multiplier=0)
        iota_bf = cpool.tile([P, 1, P], dt.bfloat16)
        nc.vector.tensor_copy(iota_bf[:], it32[:])
        ip32 = cpool.tile([P, 1], dt.int32)
        nc.gpsimd.iota(ip32[:], [[1, 1]], channel_multiplier=1)
        iota_p = cpool.tile([P, 1], dt.float32)
        nc.vector.tensor_copy(iota_p[:], ip32[:])
        ones1 = cpool.tile([1, P], dt.bfloat16)
        nc.gpsimd.memset(ones1[:], 1.0)
        ztile = cpool.tile([1, P], dt.bfloat16)
        nc.gpsimd.memset(ztile[:], 0.0)
        niota_p = cpool.tile([P, 1], dt.float32)
        nc.vector.tensor_scalar(
            out=niota_p[:], in0=iota_p[:], scalar1=-1.0, scalar2=None, op0=OP.mult,
        )
        # ---- edge index/dst tables (loaded early, scalar-engine DMA) ----
        i1lo_t = cpool.tile([P, n1lo * 8], dt.int16)
        nc.scalar.dma_start(i1lo_t[:], i1lo_d[:])
        i1hi_t = cpool.tile([P, max(n1hi, 1) * 8], dt.int16)
        nc.scalar.dma_start(i1hi_t[:], i1hi_d[:])
        i2lo_t = cpool.tile([P, n2lo * 8], dt.int16)
        nc.scalar.dma_start(i2lo_t[:], i2lo_d[:])
        dp1_bf = cpool.tile([P, ct1, 1], dt.bfloat16)
        nc.scalar.dma_start(dp1_bf[:, :, 0], dp1_d[:])
        dp2_bf = cpool.tile([P, ct2, 1], dt.bfloat16)
        nc.scalar.dma_start(dp2_bf[:, :, 0], dp2_d[:])
        pp2_t = cpool.tile([P, ct2], dt.bfloat16)
        nc.scalar.dma_start(pp2_t[:], pp2_d[:])
        b1_t = cpool.tile([P, FEAT], dt.float32)
        nc.sync.dma_start(b1_t[:], b1_d[:].partition_broadcast(P))
        w2b = cpool.tile([P, FEAT], dt.float32)
        nc.sync.dma_start(w2b[:], w2_d[:].partition_broadcast(P))
        ad_loc = cpool.tile([P, TILES, HEADS], dt.bfloat16)
        xw2loc = cpool.tile([P, TILES], dt.float32)
        out_sb = cpool.tile([P, TILES], dt.float32)

        xw2_bounce = dpool.tile([SHARD], dt.bfloat16)
        xw2_pad = dpool.tile([NPACK + P], dt.bfloat16)
        nc.sync.dma_start(
            xw2_pad[NPACK:NPACK + P].rearrange("(o k) -> o k", o=1), ztile[:1, :]
        )

        # =================== phase 1: projection ===================
        with (
            tc.tile_pool(name="p1", bufs=3) as pool,
            tc.tile_pool(name="p1w", bufs=1) as wpool,
            tc.tile_pool(name="p1ps", bufs=4, space="PSUM") as psp,
        ):
            w1a_t = wpool.tile([P, 2, W1AC], dt.bfloat16)
            nc.sync.dma_start(w1a_t[:], w1a_d[:].rearrange("c p f -> p c f"))

            for b in range(NBATCH):
                xt_b = pool.tile([P, 2, NB * P], dt.bfloat16)
                for cc in range(2):
                    nc.sync.dma_start(
                        xt_b[:, cc, :], xT_d[cc * P:(cc + 1) * P, b * NB * P:(b + 1) * NB * P]
                    )
                stage = pool.tile([P, NB, ROWLEN], dt.bfloat16)
                for i in range(NB):
                    t = b * NB + i
                    xw_ps = psp.tile([P, W1AC], dt.float32)
                    for cc in range(2):
                        nc.tensor.matmul(
                            xw_ps[:],
                            lhsT=xt_b[:, cc, i * P:(i + 1) * P],
                            rhs=w1a_t[:, cc, :],
                            start=(cc == 0), stop=(cc == 1),
                        )
                    if i % 2 == 0:
                        nc.vector.tensor_copy(stage[:, i, 0:FEAT + HEADS], xw_ps[:, 0:FEAT + HEADS])
                    else:
                        nc.scalar.copy(stage[:, i, 0:FEAT + HEADS], xw_ps[:, 0:FEAT + HEADS])
                    if t < TILES:
                        nc.vector.tensor_copy(ad_loc[:, t, :], xw_ps[:, FEAT + HEADS:W1AC])
                if b < NBLO:
                    dst = XWlo[b * NB * P:(b + 1) * NB * P, :]
                else:
                    dst = XWhi[(b - NBLO) * NB * P:(b - NBLO + 1) * NB * P, :]
                nc.sync.dma_start(dst.rearrange("(p i) f -> p i f", p=P), stage[:])

        # =================== phase 2: layer-1 edges ===================
        with (
            tc.tile_pool(name="g1", bufs=3) as gpool,
            tc.tile_pool(name="e1", bufs=3) as pool,
            tc.tile_pool(name="e1s", bufs=2, space="PSUM") as psS,
            tc.tile_pool(name="e1d", bufs=3, space="PSUM") as psD,
            tc.tile_pool(name="e1a", bufs=3, space="PSUM") as psA,
        ):
            pos = plo = phi = 0
            for t in range(TILES):
                c_lo, c_hi = int(sched1[t, 0]), int(sched1[t, 1])
                nch = c_lo + c_hi
                S_ps = psS.tile([P, FEAT + HEADS], dt.float32)
                done = 0
                while done < nch:
                    cR = min(CMAX, nch - done)
                    r0 = pos + done
                    G = gpool.tile([P, CMAX, ROWLEN], dt.bfloat16, tag="G")
                    lo_a, lo_b = done, min(done + cR, c_lo)
                    for g0 in range(lo_a, lo_b, 8):
                        g1 = min(g0 + 8, lo_b)
                        nn = g1 - g0
                        nc.gpsimd.dma_gather(
                            out_ap=G[:, g0 - done:g1 - done, :],
                            in_ap=XWlo[:, :],
                            idxs_ap=i1lo_t[:, (plo + g0) * 8:(plo + g1) * 8],
                            num_idxs=nn * P, num_idxs_reg=nn * P, elem_size=ROWLEN,
                            queue_num=next_q(),
                        )
                    hi_a, hi_b = max(done, c_lo), done + cR
                    for g0 in range(hi_a, hi_b, 8):
                        g1 = min(g0 + 8, hi_b)
                        nn = g1 - g0
                        nc.gpsimd.dma_gather(
                            out_ap=G[:, g0 - done:g1 - done, :],
                            in_ap=XWhi[:, :],
                            idxs_ap=i1hi_t[:, (phi + g0 - c_lo) * 8:(phi + g1 - c_lo) * 8],
                            num_idxs=nn * P, num_idxs_reg=nn * P, elem_size=ROWLEN,
                            queue_num=next_q(),
                        )
                    # ---- one-hots ----
                    T1 = pool.tile([P, CMAX, P], dt.bfloat16, tag="T1")
                    nc.vector.tensor_tensor(
                        out=T1[:, :cR, :], in0=iota_bf[:].to_broadcast((P, cR, P)),
                        in1=dp1_bf[:, r0:r0 + cR, :].to_broadcast((P, cR, P)),
                        op=OP.is_equal,
                    )
                    drow = pool.tile([1, CMAX * P], dt.bfloat16, tag="drow")
                    nc.sync.dma_start(drow[:1, :cR * P], dr1_d[:1, r0 * P:(r0 + cR) * P])
                    # T2 one-hot via scalar engine: relu(1 - |dbc - p|)
                    T2 = pool.tile([P, CMAX * P], dt.bfloat16, tag="T2")
                    for s0 in range(0, cR * P, 512):
                        s1 = min(s0 + 512, cR * P)
                        dbc = psD.tile([P, 512], dt.float32, tag="dbc")
                        nc.tensor.matmul(
                            dbc[:, :s1 - s0], lhsT=ones1[:], rhs=drow[:1, s0:s1],
                            start=True, stop=True,
                        )
                        nc.scalar.activation(
                            T2[:, s0:s1], dbc[:, :s1 - s0], AF.Abs, bias=niota_p[:],
                        )
                    nc.scalar.activation(
                        T2[:, :cR * P], T2[:, :cR * P], AF.Relu, bias=1.0, scale=-1.0,
                    )
                    # ---- alpha_dst per edge ----
                    ad_ps = psA.tile([P, CMAX * HEADS], dt.float32, tag="adps")
                    for j in range(cR):
                        nc.tensor.matmul(
                            ad_ps[:, j * HEADS:(j + 1) * HEADS],
                            lhsT=T2[:, j * P:(j + 1) * P], rhs=ad_loc[:, t, :],
                            start=True, stop=True,
                        )
                    # ---- p = exp(lrelu(als + ad)) ----
                    ev = pool.tile([P, CMAX, HEADS], dt.float32, tag="ev")
                    nc.vector.tensor_tensor(
                        out=ev[:, :cR, :], in0=G[:, :cR, FEAT:FEAT + HEADS],
                        in1=ad_ps[:, :cR * HEADS].rearrange("p (c h) -> p c h", h=HEADS),
                        op=OP.add,
                    )
                    lrl = pool.tile([P, CMAX * HEADS], dt.float32, tag="lrl")
                    nc.vector.scalar_tensor_tensor(
                        out=lrl[:, :cR * HEADS],
                        in0=ev[:, :cR, :].rearrange("p c h -> p (c h)"),
                        scalar=NEG_SLOPE,
                        in1=ev[:, :cR, :].rearrange("p c h -> p (c h)"),
                        op0=OP.mult, op1=OP.max,
                    )
                    pv = pool.tile([P, CMAX, HEADS], dt.bfloat16, tag="pv")
                    nc.scalar.activation(
                        pv[:, :cR, :].rearrange("p c h -> p (c h)"),
                        lrl[:, :cR * HEADS], AF.Exp,
                    )
                    # ---- MSGP = [p*feat | p] ----
                    MSGP = pool.tile([P, CMAX, FEAT + HEADS], dt.bfloat16, tag="MSGP")
                    nc.vector.tensor_tensor(
                        out=MSGP[:, :cR, 0:FEAT].rearrange("p c (h f) -> p c h f", h=HEADS),
                        in0=G[:, :cR, 0:FEAT].rearrange("p c (h f) -> p c h f", h=HEADS),
                        in1=pv[:, :cR, :].rearrange("p c (h o) -> p c h o", o=1).to_broadcast((P, cR, HEADS, HID)),
                        op=OP.mult,
                    )
                    nc.scalar.copy(MSGP[:, :cR, FEAT:FEAT + HEADS], pv[:, :cR, :])
                    # ---- segment matmul ----
                    for j in range(cR):
                        nc.tensor.matmul(
                            S_ps[:], lhsT=T1[:, j, :], rhs=MSGP[:, j, :],
                            start=(done + j == 0), stop=(done + j == nch - 1),
                        )
                    done += cR
                plo += c_lo
                phi += c_hi
                pos += nch
                # ---- tile epilogue: h1 = relu(S/denom + b1); xw2 = h1.w2 ----
                den = pool.tile([P, HEADS], dt.float32, tag="den")
                nc.vector.tensor_scalar(
                    out=den[:], in0=S_ps[:, FEAT:FEAT + HEADS],
                    scalar1=EPS, scalar2=None, op0=OP.add,
                )
                rec = pool.tile([P, HEADS, 1], dt.float32, tag="rec")
                nc.vector.reciprocal(rec[:, :, 0], den[:])
                h1a = pool.tile([P, FEAT], dt.float32, tag="h1a")
                nc.vector.tensor_tensor(
                    out=h1a[:].rearrange("p (h f) -> p h f", h=HEADS),
                    in0=S_ps[:, 0:FEAT].rearrange("p (h f) -> p h f", h=HEADS),
                    in1=rec[:].to_broadcast((P, HEADS, HID)), op=OP.mult,
                )
                nc.vector.tensor_tensor(out=h1a[:], in0=h1a[:], in1=b1_t[:], op=OP.add)
                h1bf = pool.tile([P, FEAT], dt.float32, tag="h1bf")
                nc.scalar.activation(h1bf[:], h1a[:], AF.Relu)
                hw = pool.tile([P, FEAT], dt.float32, tag="hw")
                nc.vector.tensor_tensor(out=hw[:], in0=h1bf[:], in1=w2b[:], op=OP.mult)
                nc.vector.tensor_reduce(
                    out=xw2loc[:, t:t + 1], in_=hw[:],
                    axis=mybir.AxisListType.X, op=OP.add,
                )

            # ---- allgather xw2; build shifted-replica table ----
            xw2bf = pool.tile([P, TILES], dt.bfloat16, tag="xw2bf")
            nc.vector.tensor_copy(xw2bf[:], xw2loc[:])
            nc.sync.dma_start(xw2_bounce[:].rearrange("(t p) -> p t", p=P), xw2bf[:])
            nc.gpsimd.collective_compute(
                "AllGather", mybir.AluOpType.bypass,
                replica_groups=[list(range(NCORES))],
                ins=[xw2_bounce[:].opt()], outs=[xw2_pad[0:NPACK].opt()],
            )

        # =================== phase 3: layer-2 edges ===================
        with (
            tc.tile_pool(name="g2", bufs=3) as gpool,
            tc.tile_pool(name="e2", bufs=3) as pool,
            tc.tile_pool(name="e2s", bufs=2, space="PSUM") as psS,
            tc.tile_pool(name="e2d", bufs=3, space="PSUM") as psD,
            tc.tile_pool(name="e2x", bufs=3, space="PSUM") as psA,
        ):
            # 2-shift replica table: row j2*392+q = xw2_pad[128q+2*j2 : +128]
            engs = [nc.sync, nc.scalar]
            for j2 in range(NSHIFT):
                engs[j2 % 2].dma_start(
                    XW2S[j2 * NTILES_GLOBAL:(j2 + 1) * NTILES_GLOBAL, :],
                    xw2_pad[2 * j2:2 * j2 + NPACK].rearrange("(q k) -> q k", k=P),
                )

            pos = plo = phi = 0
            for t in range(TILES):
                c_lo, c_hi = int(sched2[t, 0]), int(sched2[t, 1])
                nch = c_lo + c_hi
                S2_ps = psS.tile([P, 2], dt.float32)
                xd_bf = pool.tile([P, 1], dt.bfloat16, tag="xdbf")
                nc.vector.tensor_copy(xd_bf[:], xw2loc[:, t:t + 1])
                done = 0
                while done < nch:
                    cR = min(CMAX, nch - done)
                    r0 = pos + done
                    XS = gpool.tile([P, CMAX, P], dt.bfloat16, tag="XS")
                    lo_a, lo_b = done, min(done + cR, c_lo)
                    for g0 in range(lo_a, lo_b, 8):
                        g1 = min(g0 + 8, lo_b)
                        nn = g1 - g0
                        nc.gpsimd.dma_gather(
                            out_ap=XS[:, g0 - done:g1 - done, :],
                            in_ap=XW2S[:, :],
                            idxs_ap=i2lo_t[:, (plo + g0) * 8:(plo + g1) * 8],
                            num_idxs=nn * P, num_idxs_reg=nn * P, elem_size=P,
                            queue_num=next_q(),
                        )
                    # per-edge src scalar: parity-select between cols 0/1
                    xsv = pool.tile([P, CMAX], dt.float32, tag="xsv")
                    nc.vector.tensor_tensor(
                        out=xsv[:, :cR],
                        in0=XS[:, :cR, 1:2].rearrange("p c o -> p (c o)"),
                        in1=XS[:, :cR, 0:1].rearrange("p c o -> p (c o)"),
                        op=OP.subtract,
                    )
                    nc.vector.tensor_tensor(
                        out=xsv[:, :cR], in0=xsv[:, :cR],
                        in1=pp2_t[:, r0:r0 + cR], op=OP.mult,
                    )
                    nc.vector.tensor_tensor(
                        out=xsv[:, :cR], in0=xsv[:, :cR],
                        in1=XS[:, :cR, 0:1].rearrange("p c o -> p (c o)"),
                        op=OP.add,
                    )
                    xs = xsv[:, :cR]
                    # one-hots
                    T1 = pool.tile([P, CMAX, P], dt.bfloat16, tag="T12")
                    nc.vector.tensor_tensor(
                        out=T1[:, :cR, :], in0=iota_bf[:].to_broadcast((P, cR, P)),
                        in1=dp2_bf[:, r0:r0 + cR, :].to_broadcast((P, cR, P)),
                        op=OP.is_equal,
                    )
                    drow = pool.tile([1, CMAX * P], dt.bfloat16, tag="drow2")
                    nc.sync.dma_start(drow[:1, :cR * P], dr2_d[:1, r0 * P:(r0 + cR) * P])
                    T2 = pool.tile([P, CMAX * P], dt.bfloat16, tag="T22")
                    for s0 in range(0, cR * P, 512):
                        s1 = min(s0 + 512, cR * P)
                        dbc = psD.tile([P, 512], dt.float32, tag="dbc2")
                        nc.tensor.matmul(
                            dbc[:, :s1 - s0], lhsT=ones1[:], rhs=drow[:1, s0:s1],
                            start=True, stop=True,
                        )
                        nc.scalar.activation(
                            T2[:, s0:s1], dbc[:, :s1 - s0], AF.Abs, bias=niota_p[:],
                        )
                    nc.scalar.activation(
                        T2[:, :cR * P], T2[:, :cR * P], AF.Relu, bias=1.0, scale=-1.0,
                    )
                    # xd per edge
                    xd_ps = psA.tile([P, CMAX], dt.float32, tag="xdps")
                    for j in range(cR):
                        nc.tensor.matmul(
                            xd_ps[:, j:j + 1],
                            lhsT=T2[:, j * P:(j + 1) * P], rhs=xd_bf[:],
                            start=True, stop=True,
                        )
                    # e2 = a2s*xs + a2d*xd ; p2 = exp(lrelu(e2))
                    e2 = pool.tile([P, CMAX], dt.float32, tag="e2t")
                    nc.vector.tensor_scalar(
                        out=e2[:, :cR], in0=xd_ps[:, :cR], scalar1=a2d, scalar2=None, op0=OP.mult,
                    )
                    nc.vector.scalar_tensor_tensor(
                        out=e2[:, :cR], in0=xs, scalar=a2s,
                        in1=e2[:, :cR], op0=OP.mult, op1=OP.add,
                    )
                    lr2 = pool.tile([P, CMAX], dt.float32, tag="lr2")
                    nc.vector.scalar_tensor_tensor(
                        out=lr2[:, :cR], in0=e2[:, :cR],
                        scalar=NEG_SLOPE, in1=e2[:, :cR],
                        op0=OP.mult, op1=OP.max,
                    )
                    p2 = pool.tile([P, CMAX], dt.float32, tag="p2t")
                    nc.scalar.activation(p2[:, :cR], lr2[:, :cR], AF.Exp)
                    MS2 = pool.tile([P, CMAX, 2], dt.bfloat16, tag="MS2")
                    nc.vector.tensor_tensor(
                        out=MS2[:, :cR, 0], in0=p2[:, :cR], in1=xs, op=OP.mult,
                    )
                    nc.vector.tensor_copy(MS2[:, :cR, 1], p2[:, :cR])
                    for j in range(cR):
                        nc.tensor.matmul(
                            S2_ps[:], lhsT=T1[:, j, :], rhs=MS2[:, j, :],
                            start=(done + j == 0), stop=(done + j == nch - 1),
                        )
                    done += cR
                plo += c_lo
                phi += c_hi
                pos += nch
                den2 = pool.tile([P, 1], dt.float32, tag="den2")
                nc.vector.tensor_scalar(
                    out=den2[:], in0=S2_ps[:, 1:2], scalar1=EPS, scalar2=None, op0=OP.add,
                )
                rec2 = pool.tile([P, 1], dt.float32, tag="rec2")
                nc.vector.reciprocal(rec2[:], den2[:])
                nc.vector.scalar_tensor_tensor(
                    out=out_sb[:, t:t + 1], in0=S2_ps[:, 0:1], scalar=b2,
                    in1=rec2[:], op0=OP.bypass, op1=OP.mult,
                )
            nc.vector.tensor_scalar(
                out=out_sb[:], in0=out_sb[:], scalar1=b2, scalar2=None, op0=OP.add,
            )
            nc.sync.dma_start(out_d[:].rearrange("(t p) o -> p (t o)", p=P), out_sb[:])

    nc.finalize()
    return nc


LAST_EXEC_NS = None


def kernel(**inputs):
    import os
    from concourse.bass_utils import run_bass_kernel_spmd

    shared, cores = _host_arrays(inputs)
    sched1, sched2 = shared["sched1"], shared["sched2"]
    n1lo, n1hi = int(sched1[:, 0].sum()), int(sched1[:, 1].sum())
    n2lo, n2hi = int(sched2[:, 0].sum()), int(sched2[:, 1].sum())
    ct1, ct2 = n1lo + n1hi, n2lo + n2hi

    nc = _build_program(shared, n1lo, n1hi, ct1, n2lo, n2hi, ct2)

    in_maps = []
    for c in range(NCORES):
        m = cores[c]
        in_maps.append({
            "xT": np.asarray(m["xT"]),
            "w1a": np.asarray(shared["w1a_bf"]),
            "b1": shared["b1"],
            "w2row": shared["w2row"],
            "idx1_lo": m["idx1_lo"],
            "idx1_hi": m["idx1_hi"],
            "idx2_lo": m["idx2_lo"],
            "idx2_hi": m["idx2_hi"],
            "dstpc1": m["dstpc1"],
            "dstrow1": np.asarray(m["dstrow1"]),
            "dstpc2": m["dstpc2"],
            "dstrow2": np.asarray(m["dstrow2"]),
            "parpc2": np.asarray(m["parpc2"]),
        })

    trace = os.environ.get("GAT_TRACE", "0") == "1"
    res = run_bass_kernel_spmd(nc, in_maps, core_ids=list(range(NCORES)), trace=trace)
    global LAST_EXEC_NS
    LAST_EXEC_NS = res.exec_time_ns
    out = np.concatenate([res.results[c]["out"] for c in range(NCORES)], axis=0)
    return out[:N].astype(np.float32)


if __name__ == "__main__":
    pass
